# revision 14
# baseline (speedup 1.0000x reference)
"""Trainium2 Bass kernel for nn_Attention_16441134809282 (sparse sliding-window GQA).

Self-contained: hardcodes shapes from the problem spec.
Sharding: 8 cores; core c owns q-heads {2c, 2c+1} and kv-head c (tensor
parallel over heads). Each core computes a partial output [T, D] (its heads'
contribution through w_out); the host sums the 8 partials.

All matmuls run in float32r (TRN2 reduced-precision fp32 path, ~1 cyc/row at
free-dim >= 256, ~1e-4 rel error) with fp32 PSUM accumulation.
"""
import os

import numpy as np

import concourse.bass as bass  # noqa: F401
import concourse.mybir as mybir
import concourse.tile as tile
from concourse import bacc
from concourse.bass_utils import run_bass_kernel_spmd
from concourse.masks import make_identity

# problem constants
B, T, D = 1, 2048, 3072
N, K, H = 16, 8, 256
G = N // K
SOFT_CAP = 50.0
WINDOW = 1024
ROPE_BASE = 10000.0
ROPE_SCALE = 1.0
K_MASK = -2.3819763e38
EPS = 1e-6

NCORES = 8
TB = T // 128       # 16 t-blocks
DC = D // 128       # 24 d-chunks (contraction)
JQ = T // 512       # 4 query chunks of 512
DCH = D // 512      # 6 output d-chunks of 512

F32 = mybir.dt.float32
F32R = mybir.dt.float32r
F16 = mybir.dt.float16
BF16 = mybir.dt.bfloat16
AF = mybir.ActivationFunctionType
ALU = mybir.AluOpType

_PROG_CACHE: dict = {}


def _build_program(band_key, band, debug=False):
    """band: list (len JQ) of list of (kb, mask_slot or None)."""
    n_masks = max(1, sum(1 for row in band for (_, m) in row if m is not None))
    nc = bacc.Bacc("TRN2", target_bir_lowering=False, debug=False, num_devices=NCORES)

    xt_e = nc.dram_tensor("xt", [TB, 128, DC, 128], F16, kind="ExternalInput").ap()
    wq_e = nc.dram_tensor("wq", [DC // 4, 128, 4, 512], F16, kind="ExternalInput").ap()
    wkv_e = nc.dram_tensor("wkv", [DC // 4, 128, 4, 512], F16, kind="ExternalInput").ap()
    wo_e = nc.dram_tensor("wo", [DCH, 128, 4, 512], F16, kind="ExternalInput").ap()
    tabs_e = nc.dram_tensor("tabs", [TB, 128, 8, 128], F16, kind="ExternalInput").ap()
    masks_e = nc.dram_tensor("masks", [128, n_masks, 512], F16, kind="ExternalInput").ap()
    onec_e = nc.dram_tensor("onec", [128, 128], BF16, kind="ExternalInput").ap()
    out_e = nc.dram_tensor("out", [T, D], F16, kind="ExternalOutput").ap()
    dbg = {}
    if debug:
        dbg["qt"] = nc.dram_tensor("dbg_qt", [128, 4 * TB, 128], F16, kind="ExternalOutput").ap()
        dbg["kt"] = nc.dram_tensor("dbg_kt", [128, 2 * TB, 128], F16, kind="ExternalOutput").ap()
        dbg["v"] = nc.dram_tensor("dbg_v", [128, TB, 256], BF16, kind="ExternalOutput").ap()
        dbg["enc"] = nc.dram_tensor("dbg_enc", [128, 16, 512], F16, kind="ExternalOutput").ap()

    with tile.TileContext(nc) as tc:
        with (
            tc.tile_pool(name="pers", bufs=1) as pers,
            # phase-2 SBUF pools live at top level so their tiles never reuse
            # phase-1 pool space -- otherwise the first phase-2 writers wait on
            # every phase-1 reader (a full pipeline barrier at the transition)
            tc.tile_pool(name="enc", bufs=1) as encpool,
            tc.tile_pool(name="act", bufs=4) as actp,
            tc.tile_pool(name="sml", bufs=2) as sml,
            tc.tile_pool(name="ost", bufs=6) as ostp,
            # logits psum lives at banks 0-1, below phase 1's pools, so the
            # first phase-2 QK matmuls have no WAR wait on phase-1 psum release
            tc.tile_pool(name="plg", bufs=2, space="PSUM") as plgp,
        ):
            # persistent SBUF: transposed Q/K, natural V
            QT = pers.tile([128, 4 * TB, 128], F16)   # chunk = tb*4 + head*2 + hc
            KT = pers.tile([128, 2 * TB, 128], F16)   # chunk = tb*2 + hc
            V = pers.tile([128, TB, 256], BF16)        # [t%128, tb, h]

            QTv = QT[:].rearrange("p (tb hh) f -> p hh tb f", hh=4)
            KTv = KT[:].rearrange("p (tb hc) f -> p hc tb f", hc=2)

            ENC = encpool.tile([128, 16, 512], F16)  # chunk = head*8+hc*4+j
            ones_c = encpool.tile([128, 128], BF16)
            nc.gpsimd.dma_start(ones_c[:], onec_e[:])
            # tanh scale, written at end of phase 1 with a dep on the last
            # epilogue rs: gates attention ACT ops behind ALL phase-1
            # Square/Sqrt ops so the ACT table set switches exactly once
            # (sqrt_and_others -> exp_and_others) instead of thrashing
            capscale = encpool.tile([128, 1], F32)
            wo_t = [encpool.tile([128, 4, 512], F16, name=f"wo{dch}")
                    for dch in range(DCH)]
            MASKS = encpool.tile([128, n_masks, 512], F16)

            # ---------------- Phase 1 + interleaved attention start
            # j=1's attention is emitted inside the phase-1 pool scope with
            # tb15's projection work as PE filler units, so the ACT-bound
            # attention warm-up overlaps dense PE work (no transition bubble,
            # HAM stays warm). PSUM banks: plg1+pen2+pdn1+psq1+pskv1+ptrq1+
            # ptrk1 = 8 during j=1; pso(4) replaces the phase-1 pools after.
            pending = []  # filler units popped during attention stretches
            boxes = {}

            def make_outproj_units(j):
                def unit(r, dch):
                    def emit():
                        stage = ostp.tile([128, 512], F16, tag="stage",
                                          name=f"st{j}_{r}_{dch}")
                        po = boxes["pso"].tile([128, 512], F32, tag="po")
                        for hh in range(4):
                            head, hc = hh >> 1, hh & 1
                            nc.tensor.matmul(
                                po[:],
                                ENC[:, head * 8 + hc * 4 + j,
                                    r * 128:(r + 1) * 128],
                                wo_t[dch][:, hh, :],
                                start=(hh == 0), stop=(hh == 3))
                        # DVE keeps tanh->exp unblocked on ACT; in j3's
                        # post-attention drain ACT is idle, so share the copies
                        if j == 3 and dch % 2 == 1:
                            nc.scalar.activation(stage[:], po[:], AF.Identity)
                        else:
                            nc.vector.tensor_copy(stage[:], po[:])
                        tb = 4 * j + r
                        if j == 3:
                            eng = (nc.sync, nc.gpsimd, nc.scalar)[(r * DCH + dch) % 3]
                        else:
                            eng = nc.sync if (r + dch) % 2 == 0 else nc.gpsimd
                        eng.dma_start(
                            out_e[tb * 128:(tb + 1) * 128,
                                  dch * 512:(dch + 1) * 512],
                            stage[:])
                    return emit
                return [unit(r, dch) for r in range(4) for dch in range(DCH)]

            pre_ex = {}

            def qk_softmax(lgp, j, head, kb, mslot):
                lg = lgp.tile([128, 512], F32, tag="lg")
                for hc in range(2):
                    nc.tensor.matmul(
                        lg[:], KTv[:, hc, kb, :],
                        QTv[:, head * 2 + hc, 4 * j:4 * j + 4, :],
                        start=(hc == 0), stop=(hc == 1))
                th = actp.tile([128, 512], F32, tag="th")
                nc.scalar.activation(th[:], lg[:], AF.Tanh,
                                     scale=capscale[:])
                ex = actp.tile([128, 512], BF16, tag="ex")
                if mslot is not None:
                    # mask holds -20; exp(50*(th-20)) = exp(50*th-1000)
                    nc.vector.tensor_add(th[:], th[:], MASKS[:, mslot, :])
                nc.scalar.activation(ex[:], th[:], AF.Exp, scale=SOFT_CAP)
                return ex

            def attn_j(j, lgp):
                kbs = band[j]
                nkb = len(kbs)
                flushes_left = [2 * nkb]

                def pop_paced():
                    if flushes_left[0] > 0:
                        npop = -(-len(pending) // flushes_left[0])  # ceil
                        flushes_left[0] -= 1
                    else:
                        npop = len(pending)
                    for _ in range(min(npop, 3, len(pending))):
                        pending.pop(0)()

                for head in range(2):
                    enc_ps = boxes["pen"].tile([128, 2, 512], F32, tag="enc")
                    den_ps = boxes["pdn"].tile([128, 512], F32, tag="den")
                    pend_av = None  # exp tile awaiting denom+AV, 1-kb lag

                    def flush_av(i, kb, ex):
                        nc.tensor.matmul(den_ps[:], ones_c[:], ex[:],
                                         start=(i == 0), stop=(i == nkb - 1))
                        for hc in range(2):
                            nc.tensor.matmul(
                                enc_ps[:, hc, :],
                                V[:, kb, hc * 128:(hc + 1) * 128], ex[:],
                                start=(i == 0), stop=(i == nkb - 1))

                    for i, (kb, mslot) in enumerate(kbs):
                        ex = pre_ex.pop((j, head, i), None)
                        if ex is None:
                            ex = qk_softmax(lgp, j, head, kb, mslot)
                        if pend_av is not None:
                            flush_av(*pend_av)
                            pop_paced()
                        pend_av = (i, kb, ex)
                    flush_av(*pend_av)
                    pop_paced()

                    # fold 1/denominator into enc (den_ps rows all identical)
                    rep_rec = sml.tile([128, 512], F32, tag="rep_rec")
                    nc.vector.reciprocal_approx_fast(rep_rec[:], den_ps[:])
                    for hc in range(2):
                        nc.vector.tensor_mul(ENC[:, head * 8 + hc * 4 + j, :],
                                             enc_ps[:, hc, :], rep_rec[:])

                # drain leftover units, then arm this j's out-proj units
                for u in pending:
                    u()
                pending[:] = make_outproj_units(j)

            if True:
                with (
                    tc.tile_pool(name="wts", bufs=1) as wts,
                    tc.tile_pool(name="xs", bufs=3) as xsp,
                    tc.tile_pool(name="tab", bufs=2) as tabp,
                    tc.tile_pool(name="rot", bufs=2) as rotp,
                    tc.tile_pool(name="wk", bufs=2) as wk,
                    tc.tile_pool(name="psq", bufs=2, space="PSUM") as psqp,
                    tc.tile_pool(name="pskv", bufs=2, space="PSUM") as pskvp,
                    tc.tile_pool(name="ptrq", bufs=1, space="PSUM") as ptrqp,
                    tc.tile_pool(name="ptrk", bufs=1, space="PSUM") as ptrkp,
                ):
                    wq_c = [wts.tile([128, 4, 512], F16, tag=f"wq{g}", name=f"wq{g}")
                            for g in range(DC // 4)]
                    wkv_c = [wts.tile([128, 4, 512], F16, tag=f"wkv{g}", name=f"wkv{g}")
                             for g in range(DC // 4)]
                    # weights ride the scalar+gpsimd queues in first-use
                    # order; the sync queue carries ONLY the xt strips so
                    # tb1/tb2 matmuls are never stuck behind weight transfers
                    nc.scalar.dma_start(wq_c[0][:, 0:1, :], wq_e[0, :, 0:1, :])
                    nc.scalar.dma_start(wq_c[0][:, 1:4, :], wq_e[0, :, 1:4, :])
                    nc.scalar.dma_start(wq_c[1][:], wq_e[1])
                    nc.gpsimd.dma_start(wq_c[2][:], wq_e[2])
                    nc.gpsimd.dma_start(wq_c[3][:], wq_e[3])
                    nc.scalar.dma_start(wq_c[4][:], wq_e[4])
                    nc.gpsimd.dma_start(wq_c[5][:], wq_e[5])
                    for g in range(3):
                        nc.scalar.dma_start(wkv_c[g][:], wkv_e[g])
                    for g in range(3, 6):
                        nc.gpsimd.dma_start(wkv_c[g][:], wkv_e[g])

                    ident = wts.tile([128, 128], F16)
                    make_identity(nc, ident[:])

                    def rsqrt_of_meansq(src_ap, nfree, tag):
                        """rs = rsqrt(mean(src^2) + EPS), per partition row."""
                        scr = wk.tile([128, nfree], F32, tag="sq_scr")
                        ssq = wk.tile([128, 1], F32, tag=tag + "_ssq")
                        nc.scalar.activation(scr[:], src_ap, AF.Square,
                                             accum_out=ssq[:])
                        var = wk.tile([128, 1], F32, tag=tag + "_var")
                        nc.vector.tensor_scalar(var[:], ssq[:], 1.0 / nfree,
                                                EPS, ALU.mult, ALU.add)
                        rec = wk.tile([128, 1], F32, tag=tag + "_rec")
                        nc.vector.reciprocal(rec[:], var[:])
                        rs = wk.tile([128, 1], F32, tag=tag + "_rs")
                        nc.scalar.activation(rs[:], rec[:], AF.Sqrt)
                        st["last_rs"] = rs
                        return rs

                    def rope_norm(dst, psrc, off, rs, tabs, tb0):
                        f = psrc[:, off:off + 128]
                        sc = psrc[:, off + 128:off + 256]
                        dst_f = dst[:, off:off + 128]
                        dst_s = dst[:, off + 128:off + 256]
                        t2 = wk.tile([128, 128], F16, tag="rope_t2")
                        nc.vector.scalar_tensor_tensor(
                            dst_f, f, rs[:], tabs[:, tb0 + 0, :], ALU.mult, ALU.mult)
                        nc.vector.scalar_tensor_tensor(
                            t2[:], sc, rs[:], tabs[:, tb0 + 1, :], ALU.mult, ALU.mult)
                        nc.vector.tensor_sub(dst_f, dst_f, t2[:])
                        nc.vector.scalar_tensor_tensor(
                            dst_s, sc, rs[:], tabs[:, tb0 + 2, :], ALU.mult, ALU.mult)
                        nc.vector.scalar_tensor_tensor(
                            t2[:], f, rs[:], tabs[:, tb0 + 3, :], ALU.mult, ALU.mult)
                        nc.vector.tensor_add(dst_s, dst_s, t2[:])

                    def emit_transposes(pend):
                        tb, qrot, krot = pend
                        ptr = ptrqp.tile([128, 4, 128], F16, tag="ptrq")
                        for c in range(4):
                            nc.tensor.transpose(ptr[:, c, :],
                                                qrot[:, c * 128:(c + 1) * 128],
                                                ident[:])
                        nc.vector.tensor_copy(QT[:, tb * 4:tb * 4 + 4, :], ptr[:])
                        ptr2 = ptrkp.tile([128, 2, 128], F16, tag="ptrk")
                        for c in range(2):
                            nc.tensor.transpose(ptr2[:, c, :],
                                                krot[:, c * 128:(c + 1) * 128],
                                                ident[:])
                        nc.vector.tensor_copy(KT[:, tb * 2:tb * 2 + 2, :], ptr2[:])

                    st = {"pend": None}
                    for tb in range(TB):
                        xs = xsp.tile([128, DC, 128], F16, tag="xs",
                                      name=f"xs{tb}")
                        if tb == 0:
                            for lo, hi in ((0, 1), (1, 4), (4, 8), (8, 16), (16, 24)):
                                nc.sync.dma_start(
                                    xs[:, lo:hi, :],
                                    xt_e[tb, :, lo:hi, :])
                        else:
                            nc.sync.dma_start(xs[:], xt_e[tb])
                        tabs = tabp.tile([128, 8, 128], F16, tag="tabs",
                                         name=f"tabs{tb}")
                        nc.gpsimd.dma_start(tabs[:], tabs_e[tb])
                        if tb == 6:
                            for dch in range(DCH):
                                nc.scalar.dma_start(wo_t[dch][:], wo_e[dch])
                        if tb == 8:
                            nc.scalar.dma_start(MASKS[:], masks_e[:])

                        def u_proj(xs, wc, pool, tag, g, box):
                            def emit():
                                if g == 0:
                                    box["ps"] = pool.tile([128, 512], F32, tag=tag, name=tag)
                                ps = box["ps"]
                                for dc in range(4 * g, 4 * g + 4):
                                    nc.tensor.matmul(ps[:], xs[:, dc, :],
                                                     wc[dc // 4][:, dc % 4, :],
                                                     start=(dc == 0),
                                                     stop=(dc == DC - 1))
                            return emit

                        def u_tr(pend):
                            return lambda: emit_transposes(pend)

                        def u_epi_q(tb, tabs, qbox, rbox):
                            def emit():
                                psq = qbox["ps"]
                                qrot = rotp.tile([128, 512], F16, tag="qrot",
                                                 name=f"qrot{tb}")
                                for head in range(2):
                                    rs = rsqrt_of_meansq(
                                        psq[:, head * 256:(head + 1) * 256],
                                        256, f"q{head}")
                                    rope_norm(qrot, psq, head * 256, rs, tabs, 0)
                                rbox["qrot"] = qrot
                            return emit

                        def u_epi_kv(tb, tabs, kvbox, rbox):
                            def emit():
                                pskv = kvbox["ps"]
                                krot = rotp.tile([128, 256], F16, tag="krot",
                                                 name=f"krot{tb}")
                                rs = rsqrt_of_meansq(pskv[:, 0:256], 256, "k")
                                rope_norm(krot, pskv, 0, rs, tabs, 4)
                                rs = rsqrt_of_meansq(pskv[:, 256:512], 256, "v")
                                nc.vector.tensor_scalar_mul(
                                    V[:, tb, :], pskv[:, 256:512], rs[:])
                                st["pend"] = (tb, rbox["qrot"], krot)
                            return emit

                        # kv lags q by one tb: tb0's matmuls are gated only
                        # by wq (3 MB), not wq+wkv, shrinking the startup stall
                        qbox, kvbox, rbox = {}, {}, {}
                        for g in range(6):
                            u_proj(xs, wq_c, psqp, "psq", g, qbox)()
                        if tb >= 1:
                            pxs, ptabs, pq, pkv, pr = prev_tb
                            for g in range(6):
                                u_proj(pxs, wkv_c, pskvp, "pskv", g, pkv)()
                            if st["pend"]:
                                emit_transposes(st["pend"])
                            u_epi_q(tb - 1, ptabs, pq, pr)()
                            u_epi_kv(tb - 1, ptabs, pkv, pr)()
                        prev_tb = (xs, tabs, qbox, kvbox, rbox)
                    pxs, ptabs, pq, pkv, pr = prev_tb
                    for g in range(6):
                        u_proj(pxs, wkv_c, pskvp, "pskv", g, pkv)()
                    emit_transposes(st["pend"])
                    u_epi_q(TB - 1, ptabs, pq, pr)()
                    u_epi_kv(TB - 1, ptabs, pkv, pr)()
                    emit_transposes(st["pend"])
                    # capscale = 1/SOFT_CAP, with a data dep on the final
                    # epilogue rs so every attention Tanh queues after all
                    # phase-1 Square/Sqrt ACT work (one table switch total)
                    nc.vector.tensor_scalar(capscale[:], st["last_rs"][:],
                                            0.0, 1.0 / SOFT_CAP,
                                            ALU.mult, ALU.add)

                if debug:
                    nc.sync.dma_start(dbg["qt"][:], QT[:])
                    nc.sync.dma_start(dbg["kt"][:], KT[:])
                    nc.sync.dma_start(dbg["v"][:], V[:])

                with (
                    tc.tile_pool(name="pen", bufs=1, space="PSUM") as penp,
                    tc.tile_pool(name="pdn", bufs=1, space="PSUM") as pdnp,
                    tc.tile_pool(name="pso", bufs=3, space="PSUM") as psop,
                ):
                    boxes["pso"] = psop
                    boxes["pen"] = penp
                    boxes["pdn"] = pdnp
                    for j in [0, 1, 2, 3]:
                        attn_j(j, plgp)
                    for u in pending:
                        u()
            if debug:
                nc.sync.dma_start(dbg["enc"][:], ENC[:])

    nc.compile()
    return nc


def _host_prepare(x, segment_pos, attn_mask, w_q, w_kv, w_out, q_scale, k_scale):
    x2 = np.ascontiguousarray(np.asarray(x, np.float32).reshape(T, D))
    pos = np.asarray(segment_pos).reshape(T).astype(np.int64)
    am = np.asarray(attn_mask).reshape(T, T).astype(bool)

    # rope tables, fp32 like the reference
    half = H // 2
    fraction = (2.0 * np.arange(half, dtype=np.float32) / np.float32(H)).astype(np.float32)
    timescale = (np.float32(ROPE_BASE) ** fraction).astype(np.float32)
    sinusoid = (pos.astype(np.float32)[:, None] / timescale[None, :]) / np.float32(ROPE_SCALE)
    sin = np.sin(sinusoid).astype(np.float32)
    cos = np.cos(sinusoid).astype(np.float32)
    qsf = (1.0 + np.asarray(q_scale, np.float32))
    ksf = (1.0 + np.asarray(k_scale, np.float32))
    # tabs[t, 0..7, i]: q: cos*qsf_f, sin*qsf_s, cos*qsf_s, sin*qsf_f; then k
    tabs = np.empty((T, 8, half), np.float32)
    tabs[:, 0] = cos * qsf[None, :half]
    tabs[:, 1] = sin * qsf[None, half:]
    tabs[:, 2] = cos * qsf[None, half:]
    tabs[:, 3] = sin * qsf[None, :half]
    tabs[:, 4] = cos * ksf[None, :half]
    tabs[:, 5] = sin * ksf[None, half:]
    tabs[:, 6] = cos * ksf[None, half:]
    tabs[:, 7] = sin * ksf[None, :half]
    tabs = np.ascontiguousarray(tabs.reshape(TB, 128, 8, half)).astype(np.float16)

    # combined mask -> band structure + additive mask tiles (transposed [k, q])
    sliding = (pos[None, :] > pos[:, None] - WINDOW) & (pos[None, :] < pos[:, None] + WINDOW)
    comb = am & sliding
    band = []
    mask_list = []
    for j in range(JQ):
        row = []
        sub_q = comb[j * 512:(j + 1) * 512]
        for kb in range(T // 128):
            sub = sub_q[:, kb * 128:(kb + 1) * 128]
            if not sub.any():
                continue
            if sub.all():
                row.append((kb, None))
            else:
                mask_list.append(
                    np.where(sub.T, np.float32(0.0), np.float32(-20.0)))
                row.append((kb, len(mask_list) - 1))
        band.append(row)
    masks = (np.ascontiguousarray(np.stack(mask_list, axis=1).astype(np.float16))
             if mask_list else np.zeros((128, 1, 512), np.float16))

    # x transposed + tiled: xt[tb, p, dc, t] = x2[tb*128+t, dc*128+p]
    xt = np.ascontiguousarray(
        x2.reshape(TB, 128, DC, 128).transpose(0, 3, 2, 1)).astype(np.float16)

    tabs_full = tabs  # [TB, 128, 8, 128] with p = t % 128? fix below
    return x2, xt, tabs_full, band, masks


def kernel(x, segment_pos, attn_mask, w_q, w_kv, w_out, q_scale, k_scale):
    x = np.asarray(x, np.float32)
    w_q = np.asarray(w_q, np.float32)
    w_kv = np.asarray(w_kv, np.float32)
    w_out = np.asarray(w_out, np.float32)
    assert x.shape == (B, T, D) and w_q.shape == (N, D, H)

    x2, xt, tabs, band, masks = _host_prepare(
        x, segment_pos, attn_mask, w_q, w_kv, w_out, q_scale, k_scale)

    band_key = tuple(tuple(row) for row in band)
    debug = bool(int(os.environ.get("BASS_ATTN_DEBUG", "0")))
    cache_key = (band_key, debug)
    if cache_key not in _PROG_CACHE:
        _PROG_CACHE[cache_key] = _build_program(band_key, band, debug=debug)
    nc = _PROG_CACHE[cache_key]

    import ml_dtypes
    onec = np.ones((128, 128), ml_dtypes.bfloat16)

    in_maps = []
    for c in range(NCORES):
        wqc = np.concatenate([w_q[2 * c], w_q[2 * c + 1]], axis=1)  # [D, 512]
        wqc = np.ascontiguousarray(
            wqc.reshape(DC // 4, 4, 128, 512).transpose(0, 2, 1, 3)).astype(np.float16)
        wkvc = np.concatenate([w_kv[0, c], w_kv[1, c]], axis=1)     # [D, 512]
        wkvc = np.ascontiguousarray(
            wkvc.reshape(DC // 4, 4, 128, 512).transpose(0, 2, 1, 3)).astype(np.float16)
        # wo[dch, p, hh, n] = w_out[2c + head][hc*128 + p, dch*512 + n]
        woc = np.empty((DCH, 128, 4, 512), np.float32)
        for hh in range(4):
            head, hc = hh >> 1, hh & 1
            woc[:, :, hh, :] = w_out[2 * c + head][hc * 128:(hc + 1) * 128] \
                .reshape(128, DCH, 512).transpose(1, 0, 2)
        in_maps.append({
            "xt": xt, "wq": wqc, "wkv": wkvc,
            "wo": np.ascontiguousarray(woc).astype(np.float16),
            "tabs": tabs, "masks": masks, "onec": onec,
        })

    trace = bool(int(os.environ.get("BASS_ATTN_TRACE", "0")))
    res = run_bass_kernel_spmd(nc, in_maps, list(range(NCORES)), trace=trace)
    if trace and res.exec_time_ns is not None:
        print(f"HW exec time: {res.exec_time_ns} ns")
        kernel._last_exec_ns = res.exec_time_ns
        kernel._last_results = res

    total = np.zeros((T, D), np.float64)
    for c in range(NCORES):
        total += res.results[c]["out"].astype(np.float64)
    if bool(int(os.environ.get("BASS_ATTN_DEBUG", "0"))):
        kernel._dbg_results = res.results
    return total.astype(np.float32).reshape(B, T, D)



# revision 17
# speedup vs baseline: 1.0156x; 1.0156x over previous
"""Trainium2 Bass kernel for nn_Attention_16441134809282 (sparse sliding-window GQA).

Self-contained: hardcodes shapes from the problem spec.
Sharding: 8 cores; core c owns q-heads {2c, 2c+1} and kv-head c (tensor
parallel over heads). Each core computes a partial output [T, D] (its heads'
contribution through w_out); the host sums the 8 partials.

All matmuls run in float32r (TRN2 reduced-precision fp32 path, ~1 cyc/row at
free-dim >= 256, ~1e-4 rel error) with fp32 PSUM accumulation.
"""
import os

import numpy as np

import concourse.bass as bass  # noqa: F401
import concourse.mybir as mybir
import concourse.tile as tile
from concourse import bacc
from concourse.bass_utils import run_bass_kernel_spmd
from concourse.masks import make_identity

# problem constants
B, T, D = 1, 2048, 3072
N, K, H = 16, 8, 256
G = N // K
SOFT_CAP = 50.0
WINDOW = 1024
ROPE_BASE = 10000.0
ROPE_SCALE = 1.0
K_MASK = -2.3819763e38
EPS = 1e-6

NCORES = 8
TB = T // 128       # 16 t-blocks
DC = D // 128       # 24 d-chunks (contraction)
JQ = T // 512       # 4 query chunks of 512
DCH = D // 512      # 6 output d-chunks of 512

F32 = mybir.dt.float32
F32R = mybir.dt.float32r
F16 = mybir.dt.float16
BF16 = mybir.dt.bfloat16
AF = mybir.ActivationFunctionType
ALU = mybir.AluOpType

_PROG_CACHE: dict = {}


def _build_program(band_key, band, debug=False):
    """band: list (len JQ) of list of (kb, mask_slot or None)."""
    n_masks = max(1, sum(1 for row in band for (_, m) in row if m is not None))
    nc = bacc.Bacc("TRN2", target_bir_lowering=False, debug=False, num_devices=NCORES)

    xt_e = nc.dram_tensor("xt", [TB, 128, DC, 128], F16, kind="ExternalInput").ap()
    wq_e = nc.dram_tensor("wq", [DC // 4, 128, 4, 512], F16, kind="ExternalInput").ap()
    wkv_e = nc.dram_tensor("wkv", [DC // 4, 128, 4, 512], F16, kind="ExternalInput").ap()
    wo_e = nc.dram_tensor("wo", [DCH, 128, 4, 512], F16, kind="ExternalInput").ap()
    tabs_e = nc.dram_tensor("tabs", [TB, 128, 8, 128], F16, kind="ExternalInput").ap()
    masks_e = nc.dram_tensor("masks", [128, n_masks, 512], F16, kind="ExternalInput").ap()
    onec_e = nc.dram_tensor("onec", [128, 128], BF16, kind="ExternalInput").ap()
    out_e = nc.dram_tensor("out", [T, D], F16, kind="ExternalOutput").ap()
    dbg = {}
    if debug:
        dbg["qt"] = nc.dram_tensor("dbg_qt", [128, 4 * TB, 128], F16, kind="ExternalOutput").ap()
        dbg["kt"] = nc.dram_tensor("dbg_kt", [128, 2 * TB, 128], F16, kind="ExternalOutput").ap()
        dbg["v"] = nc.dram_tensor("dbg_v", [128, TB, 256], BF16, kind="ExternalOutput").ap()
        dbg["enc"] = nc.dram_tensor("dbg_enc", [128, 16, 512], F16, kind="ExternalOutput").ap()

    with tile.TileContext(nc) as tc:
        with (
            tc.tile_pool(name="pers", bufs=1) as pers,
            # phase-2 SBUF pools live at top level so their tiles never reuse
            # phase-1 pool space -- otherwise the first phase-2 writers wait on
            # every phase-1 reader (a full pipeline barrier at the transition)
            tc.tile_pool(name="enc", bufs=1) as encpool,
            tc.tile_pool(name="act", bufs=4) as actp,
            tc.tile_pool(name="sml", bufs=2) as sml,
            tc.tile_pool(name="ost", bufs=6) as ostp,
            # logits psum lives at banks 0-1, below phase 1's pools, so the
            # first phase-2 QK matmuls have no WAR wait on phase-1 psum release
            tc.tile_pool(name="plg", bufs=2, space="PSUM") as plgp,
        ):
            # persistent SBUF: transposed Q/K, natural V
            QT = pers.tile([128, 4 * TB, 128], F16)   # chunk = tb*4 + head*2 + hc
            KT = pers.tile([128, 2 * TB, 128], F16)   # chunk = tb*2 + hc
            V = pers.tile([128, TB, 256], BF16)        # [t%128, tb, h]

            QTv = QT[:].rearrange("p (tb hh) f -> p hh tb f", hh=4)
            KTv = KT[:].rearrange("p (tb hc) f -> p hc tb f", hc=2)

            ENC = encpool.tile([128, 16, 512], F16)  # chunk = head*8+hc*4+j
            ones_c = encpool.tile([128, 128], BF16)
            nc.gpsimd.dma_start(ones_c[:], onec_e[:])
            # tanh scale, written at end of phase 1 with a dep on the last
            # epilogue rs: gates attention ACT ops behind ALL phase-1
            # Square/Sqrt ops so the ACT table set switches exactly once
            # (sqrt_and_others -> exp_and_others) instead of thrashing
            capscale = encpool.tile([128, 1], F32)
            wo_t = [encpool.tile([128, 4, 512], F16, name=f"wo{dch}")
                    for dch in range(DCH)]
            MASKS = encpool.tile([128, n_masks, 512], F16)

            # ---------------- Phase 1 + interleaved attention start
            # j=1's attention is emitted inside the phase-1 pool scope with
            # tb15's projection work as PE filler units, so the ACT-bound
            # attention warm-up overlaps dense PE work (no transition bubble,
            # HAM stays warm). PSUM banks: plg1+pen2+pdn1+psq1+pskv1+ptrq1+
            # ptrk1 = 8 during j=1; pso(4) replaces the phase-1 pools after.
            pending = []  # filler units popped during attention stretches
            boxes = {}

            def make_outproj_units(j):
                def unit(r, dch):
                    def emit():
                        stage = ostp.tile([128, 512], F16, tag="stage",
                                          name=f"st{j}_{r}_{dch}")
                        po = boxes["pso"].tile([128, 512], F32, tag="po")
                        for hh in range(4):
                            head, hc = hh >> 1, hh & 1
                            nc.tensor.matmul(
                                po[:],
                                ENC[:, head * 8 + hc * 4 + j,
                                    r * 128:(r + 1) * 128],
                                wo_t[dch][:, hh, :],
                                start=(hh == 0), stop=(hh == 3))
                        # DVE keeps tanh->exp unblocked on ACT; in j3's
                        # post-attention drain ACT is idle, so share the copies
                        if j == 3 and dch % 2 == 1:
                            nc.scalar.activation(stage[:], po[:], AF.Identity)
                        else:
                            nc.vector.tensor_copy(stage[:], po[:])
                        tb = 4 * j + r
                        if j == 3:
                            eng = (nc.sync, nc.gpsimd, nc.scalar)[(r * DCH + dch) % 3]
                        else:
                            eng = nc.sync if (r + dch) % 2 == 0 else nc.gpsimd
                        eng.dma_start(
                            out_e[tb * 128:(tb + 1) * 128,
                                  dch * 512:(dch + 1) * 512],
                            stage[:])
                    return emit
                return [unit(r, dch) for r in range(4) for dch in range(DCH)]

            pre_ex = {}

            def qk_softmax(lgp, j, head, kb, mslot):
                lg = lgp.tile([128, 512], F32, tag="lg")
                for hc in range(2):
                    nc.tensor.matmul(
                        lg[:], KTv[:, hc, kb, :],
                        QTv[:, head * 2 + hc, 4 * j:4 * j + 4, :],
                        start=(hc == 0), stop=(hc == 1))
                th = actp.tile([128, 512], F32, tag="th")
                nc.scalar.activation(th[:], lg[:], AF.Tanh,
                                     scale=capscale[:])
                ex = actp.tile([128, 512], BF16, tag="ex")
                if mslot is not None:
                    # mask holds -20; exp(50*(th-20)) = exp(50*th-1000)
                    nc.vector.tensor_add(th[:], th[:], MASKS[:, mslot, :])
                nc.scalar.activation(ex[:], th[:], AF.Exp, scale=SOFT_CAP)
                return ex

            def attn_j(j, lgp):
                kbs = band[j]
                nkb = len(kbs)
                flushes_left = [2 * nkb]

                def pop_paced():
                    if flushes_left[0] > 0:
                        npop = -(-len(pending) // flushes_left[0])  # ceil
                        flushes_left[0] -= 1
                    else:
                        npop = len(pending)
                    for _ in range(min(npop, 3, len(pending))):
                        pending.pop(0)()

                for head in range(2):
                    enc_ps = boxes["pen"].tile([128, 2, 512], F32, tag="enc")
                    den_ps = boxes["pdn"].tile([128, 512], F32, tag="den")
                    pend_av = None  # exp tile awaiting denom+AV, 1-kb lag

                    def flush_av(i, kb, ex):
                        nc.tensor.matmul(den_ps[:], ones_c[:], ex[:],
                                         start=(i == 0), stop=(i == nkb - 1))
                        for hc in range(2):
                            nc.tensor.matmul(
                                enc_ps[:, hc, :],
                                V[:, kb, hc * 128:(hc + 1) * 128], ex[:],
                                start=(i == 0), stop=(i == nkb - 1))

                    for i, (kb, mslot) in enumerate(kbs):
                        ex = pre_ex.pop((j, head, i), None)
                        if ex is None:
                            ex = qk_softmax(lgp, j, head, kb, mslot)
                        if pend_av is not None:
                            flush_av(*pend_av)
                            pop_paced()
                        pend_av = (i, kb, ex)
                    flush_av(*pend_av)
                    pop_paced()

                    # fold 1/denominator into enc (den_ps rows all identical)
                    rep_rec = sml.tile([128, 512], F32, tag="rep_rec")
                    nc.vector.reciprocal_approx_fast(rep_rec[:], den_ps[:])
                    for hc in range(2):
                        nc.vector.tensor_mul(ENC[:, head * 8 + hc * 4 + j, :],
                                             enc_ps[:, hc, :], rep_rec[:])

                # drain leftover units, then arm this j's out-proj units
                for u in pending:
                    u()
                pending[:] = make_outproj_units(j)

            if True:
                with (
                    tc.tile_pool(name="wts", bufs=1) as wts,
                    tc.tile_pool(name="xs", bufs=3) as xsp,
                    tc.tile_pool(name="tab", bufs=2) as tabp,
                    tc.tile_pool(name="rot", bufs=2) as rotp,
                    tc.tile_pool(name="wk", bufs=2) as wk,
                    tc.tile_pool(name="psq", bufs=2, space="PSUM") as psqp,
                    tc.tile_pool(name="pskv", bufs=2, space="PSUM") as pskvp,
                    tc.tile_pool(name="ptrq", bufs=1, space="PSUM") as ptrqp,
                    tc.tile_pool(name="ptrk", bufs=1, space="PSUM") as ptrkp,
                ):
                    wq_c = [wts.tile([128, 4, 512], F16, tag=f"wq{g}", name=f"wq{g}")
                            for g in range(DC // 4)]
                    wkv_c = [wts.tile([128, 4, 512], F16, tag=f"wkv{g}", name=f"wkv{g}")
                             for g in range(DC // 4)]
                    qeng = [nc.scalar, nc.scalar, nc.gpsimd, nc.gpsimd]
                    nc.scalar.dma_start(wq_c[0][:, 0:2, :], wq_e[0, :, 0:2, :])
                    nc.scalar.dma_start(wq_c[0][:, 2:4, :], wq_e[0, :, 2:4, :])
                    for g in range(1, 4):
                        qeng[g].dma_start(wq_c[g][:], wq_e[g])
                    for g in range(4):
                        qeng[g].dma_start(wkv_c[g][:], wkv_e[g])

                    ident = wts.tile([128, 128], F16)
                    make_identity(nc, ident[:])
                    eps_t = wts.tile([128, 1], F32)
                    nc.gpsimd.memset(eps_t[:], EPS)

                    def rsqrt_of_meansq(src_ap, nfree, tag):
                        """rs = rsqrt(mean(src^2) + EPS), per partition row."""
                        scr = wk.tile([128, nfree], F32, tag="sq_scr")
                        ssq = wk.tile([128, 1], F32, tag=tag + "_ssq")
                        nc.scalar.activation(scr[:], src_ap, AF.Square,
                                             accum_out=ssq[:])
                        var = wk.tile([128, 1], F32, tag=tag + "_var")
                        nc.scalar.activation(var[:], ssq[:], AF.Identity,
                                             scale=1.0 / nfree, bias=eps_t[:])
                        rec = wk.tile([128, 1], F32, tag=tag + "_rec")
                        nc.vector.reciprocal(rec[:], var[:])
                        rs = wk.tile([128, 1], F32, tag=tag + "_rs")
                        nc.scalar.activation(rs[:], rec[:], AF.Sqrt)
                        st["last_rs"] = rs
                        return rs

                    def rope_norm(dst, psrc, off, rs, tabs, tb0):
                        f = psrc[:, off:off + 128]
                        sc = psrc[:, off + 128:off + 256]
                        dst_f = dst[:, off:off + 128]
                        dst_s = dst[:, off + 128:off + 256]
                        t2 = wk.tile([128, 128], F16, tag="rope_t2")
                        nc.vector.scalar_tensor_tensor(
                            dst_f, f, rs[:], tabs[:, tb0 + 0, :], ALU.mult, ALU.mult)
                        nc.vector.scalar_tensor_tensor(
                            t2[:], sc, rs[:], tabs[:, tb0 + 1, :], ALU.mult, ALU.mult)
                        nc.vector.tensor_sub(dst_f, dst_f, t2[:])
                        nc.vector.scalar_tensor_tensor(
                            dst_s, sc, rs[:], tabs[:, tb0 + 2, :], ALU.mult, ALU.mult)
                        nc.vector.scalar_tensor_tensor(
                            t2[:], f, rs[:], tabs[:, tb0 + 3, :], ALU.mult, ALU.mult)
                        nc.vector.tensor_add(dst_s, dst_s, t2[:])

                    def emit_transposes(pend):
                        tb, qrot, krot = pend
                        ptr = ptrqp.tile([128, 4, 128], F16, tag="ptrq")
                        for c in range(4):
                            nc.tensor.transpose(ptr[:, c, :],
                                                qrot[:, c * 128:(c + 1) * 128],
                                                ident[:])
                        nc.vector.tensor_copy(QT[:, tb * 4:tb * 4 + 4, :], ptr[:])
                        ptr2 = ptrkp.tile([128, 2, 128], F16, tag="ptrk")
                        for c in range(2):
                            nc.tensor.transpose(ptr2[:, c, :],
                                                krot[:, c * 128:(c + 1) * 128],
                                                ident[:])
                        nc.vector.tensor_copy(KT[:, tb * 2:tb * 2 + 2, :], ptr2[:])

                    st = {"pend": None}
                    for tb in range(TB):
                        xs = xsp.tile([128, DC, 128], F16, tag="xs",
                                      name=f"xs{tb}")
                        if tb == 0:
                            for lo, hi in ((0, 2), (2, 8), (8, 16), (16, 24)):
                                nc.sync.dma_start(
                                    xs[:, lo:hi, :],
                                    xt_e[tb, :, lo:hi, :])
                        else:
                            nc.sync.dma_start(xs[:], xt_e[tb])
                        tabs = tabp.tile([128, 8, 128], F16, tag="tabs",
                                         name=f"tabs{tb}")
                        nc.sync.dma_start(tabs[:], tabs_e[tb])
                        if tb == 0:
                            for g in (4, 5):
                                nc.sync.dma_start(wq_c[g][:], wq_e[g])
                        if tb == 1:
                            for g in (4, 5):
                                nc.sync.dma_start(wkv_c[g][:], wkv_e[g])
                        if tb == 6:
                            for dch in range(DCH):
                                nc.scalar.dma_start(wo_t[dch][:], wo_e[dch])
                        if tb == 8:
                            nc.scalar.dma_start(MASKS[:], masks_e[:])

                        def u_proj(xs, wc, pool, tag, g, box):
                            def emit():
                                if g == 0:
                                    box["ps"] = pool.tile([128, 512], F32, tag=tag, name=tag)
                                ps = box["ps"]
                                for dc in range(4 * g, 4 * g + 4):
                                    nc.tensor.matmul(ps[:], xs[:, dc, :],
                                                     wc[dc // 4][:, dc % 4, :],
                                                     start=(dc == 0),
                                                     stop=(dc == DC - 1))
                            return emit

                        def u_tr(pend):
                            return lambda: emit_transposes(pend)

                        def u_epi_q(tb, tabs, qbox, rbox):
                            def emit():
                                psq = qbox["ps"]
                                qrot = rotp.tile([128, 512], F16, tag="qrot",
                                                 name=f"qrot{tb}")
                                for head in range(2):
                                    rs = rsqrt_of_meansq(
                                        psq[:, head * 256:(head + 1) * 256],
                                        256, f"q{head}")
                                    rope_norm(qrot, psq, head * 256, rs, tabs, 0)
                                rbox["qrot"] = qrot
                            return emit

                        def u_epi_kv(tb, tabs, kvbox, rbox):
                            def emit():
                                pskv = kvbox["ps"]
                                krot = rotp.tile([128, 256], F16, tag="krot",
                                                 name=f"krot{tb}")
                                rs = rsqrt_of_meansq(pskv[:, 0:256], 256, "k")
                                rope_norm(krot, pskv, 0, rs, tabs, 4)
                                rs = rsqrt_of_meansq(pskv[:, 256:512], 256, "v")
                                nc.vector.tensor_scalar_mul(
                                    V[:, tb, :], pskv[:, 256:512], rs[:])
                                st["pend"] = (tb, rbox["qrot"], krot)
                            return emit

                        # kv lags q by one tb: tb0's matmuls are gated only
                        # by wq (3 MB), not wq+wkv, shrinking the startup stall
                        qbox, kvbox, rbox = {}, {}, {}
                        for g in range(6):
                            u_proj(xs, wq_c, psqp, "psq", g, qbox)()
                        if tb >= 1:
                            pxs, ptabs, pq, pkv, pr = prev_tb
                            for g in range(6):
                                u_proj(pxs, wkv_c, pskvp, "pskv", g, pkv)()
                            if st["pend"]:
                                emit_transposes(st["pend"])
                            u_epi_q(tb - 1, ptabs, pq, pr)()
                            u_epi_kv(tb - 1, ptabs, pkv, pr)()
                        prev_tb = (xs, tabs, qbox, kvbox, rbox)
                    pxs, ptabs, pq, pkv, pr = prev_tb
                    for g in range(6):
                        u_proj(pxs, wkv_c, pskvp, "pskv", g, pkv)()
                    emit_transposes(st["pend"])
                    u_epi_q(TB - 1, ptabs, pq, pr)()
                    u_epi_kv(TB - 1, ptabs, pkv, pr)()
                    emit_transposes(st["pend"])
                    # capscale = 1/SOFT_CAP, with a data dep on the final
                    # epilogue rs so every attention Tanh queues after all
                    # phase-1 Square/Sqrt ACT work (one table switch total)
                    nc.vector.tensor_scalar(capscale[:], st["last_rs"][:],
                                            0.0, 1.0 / SOFT_CAP,
                                            ALU.mult, ALU.add)

                if debug:
                    nc.sync.dma_start(dbg["qt"][:], QT[:])
                    nc.sync.dma_start(dbg["kt"][:], KT[:])
                    nc.sync.dma_start(dbg["v"][:], V[:])

                with (
                    tc.tile_pool(name="pen", bufs=1, space="PSUM") as penp,
                    tc.tile_pool(name="pdn", bufs=1, space="PSUM") as pdnp,
                    tc.tile_pool(name="pso", bufs=3, space="PSUM") as psop,
                ):
                    boxes["pso"] = psop
                    boxes["pen"] = penp
                    boxes["pdn"] = pdnp
                    for j in [0, 1, 2, 3]:
                        attn_j(j, plgp)
                    for u in pending:
                        u()
            if debug:
                nc.sync.dma_start(dbg["enc"][:], ENC[:])

    nc.compile()
    return nc


def _host_prepare(x, segment_pos, attn_mask, w_q, w_kv, w_out, q_scale, k_scale):
    x2 = np.ascontiguousarray(np.asarray(x, np.float32).reshape(T, D))
    pos = np.asarray(segment_pos).reshape(T).astype(np.int64)
    am = np.asarray(attn_mask).reshape(T, T).astype(bool)

    # rope tables, fp32 like the reference
    half = H // 2
    fraction = (2.0 * np.arange(half, dtype=np.float32) / np.float32(H)).astype(np.float32)
    timescale = (np.float32(ROPE_BASE) ** fraction).astype(np.float32)
    sinusoid = (pos.astype(np.float32)[:, None] / timescale[None, :]) / np.float32(ROPE_SCALE)
    sin = np.sin(sinusoid).astype(np.float32)
    cos = np.cos(sinusoid).astype(np.float32)
    qsf = (1.0 + np.asarray(q_scale, np.float32))
    ksf = (1.0 + np.asarray(k_scale, np.float32))
    # tabs[t, 0..7, i]: q: cos*qsf_f, sin*qsf_s, cos*qsf_s, sin*qsf_f; then k
    tabs = np.empty((T, 8, half), np.float32)
    tabs[:, 0] = cos * qsf[None, :half]
    tabs[:, 1] = sin * qsf[None, half:]
    tabs[:, 2] = cos * qsf[None, half:]
    tabs[:, 3] = sin * qsf[None, :half]
    tabs[:, 4] = cos * ksf[None, :half]
    tabs[:, 5] = sin * ksf[None, half:]
    tabs[:, 6] = cos * ksf[None, half:]
    tabs[:, 7] = sin * ksf[None, :half]
    tabs = np.ascontiguousarray(tabs.reshape(TB, 128, 8, half)).astype(np.float16)

    # combined mask -> band structure + additive mask tiles (transposed [k, q])
    sliding = (pos[None, :] > pos[:, None] - WINDOW) & (pos[None, :] < pos[:, None] + WINDOW)
    comb = am & sliding
    band = []
    mask_list = []
    for j in range(JQ):
        row = []
        sub_q = comb[j * 512:(j + 1) * 512]
        for kb in range(T // 128):
            sub = sub_q[:, kb * 128:(kb + 1) * 128]
            if not sub.any():
                continue
            if sub.all():
                row.append((kb, None))
            else:
                mask_list.append(
                    np.where(sub.T, np.float32(0.0), np.float32(-20.0)))
                row.append((kb, len(mask_list) - 1))
        band.append(row)
    masks = (np.ascontiguousarray(np.stack(mask_list, axis=1).astype(np.float16))
             if mask_list else np.zeros((128, 1, 512), np.float16))

    # x transposed + tiled: xt[tb, p, dc, t] = x2[tb*128+t, dc*128+p]
    xt = np.ascontiguousarray(
        x2.reshape(TB, 128, DC, 128).transpose(0, 3, 2, 1)).astype(np.float16)

    tabs_full = tabs  # [TB, 128, 8, 128] with p = t % 128? fix below
    return x2, xt, tabs_full, band, masks


def kernel(x, segment_pos, attn_mask, w_q, w_kv, w_out, q_scale, k_scale):
    x = np.asarray(x, np.float32)
    w_q = np.asarray(w_q, np.float32)
    w_kv = np.asarray(w_kv, np.float32)
    w_out = np.asarray(w_out, np.float32)
    assert x.shape == (B, T, D) and w_q.shape == (N, D, H)

    x2, xt, tabs, band, masks = _host_prepare(
        x, segment_pos, attn_mask, w_q, w_kv, w_out, q_scale, k_scale)

    band_key = tuple(tuple(row) for row in band)
    debug = bool(int(os.environ.get("BASS_ATTN_DEBUG", "0")))
    cache_key = (band_key, debug)
    if cache_key not in _PROG_CACHE:
        _PROG_CACHE[cache_key] = _build_program(band_key, band, debug=debug)
    nc = _PROG_CACHE[cache_key]

    import ml_dtypes
    onec = np.ones((128, 128), ml_dtypes.bfloat16)

    in_maps = []
    for c in range(NCORES):
        wqc = np.concatenate([w_q[2 * c], w_q[2 * c + 1]], axis=1)  # [D, 512]
        wqc = np.ascontiguousarray(
            wqc.reshape(DC // 4, 4, 128, 512).transpose(0, 2, 1, 3)).astype(np.float16)
        wkvc = np.concatenate([w_kv[0, c], w_kv[1, c]], axis=1)     # [D, 512]
        wkvc = np.ascontiguousarray(
            wkvc.reshape(DC // 4, 4, 128, 512).transpose(0, 2, 1, 3)).astype(np.float16)
        # wo[dch, p, hh, n] = w_out[2c + head][hc*128 + p, dch*512 + n]
        woc = np.empty((DCH, 128, 4, 512), np.float32)
        for hh in range(4):
            head, hc = hh >> 1, hh & 1
            woc[:, :, hh, :] = w_out[2 * c + head][hc * 128:(hc + 1) * 128] \
                .reshape(128, DCH, 512).transpose(1, 0, 2)
        in_maps.append({
            "xt": xt, "wq": wqc, "wkv": wkvc,
            "wo": np.ascontiguousarray(woc).astype(np.float16),
            "tabs": tabs, "masks": masks, "onec": onec,
        })

    trace = bool(int(os.environ.get("BASS_ATTN_TRACE", "0")))
    res = run_bass_kernel_spmd(nc, in_maps, list(range(NCORES)), trace=trace)
    if trace and res.exec_time_ns is not None:
        print(f"HW exec time: {res.exec_time_ns} ns")
        kernel._last_exec_ns = res.exec_time_ns
        kernel._last_results = res

    total = np.zeros((T, D), np.float64)
    for c in range(NCORES):
        total += res.results[c]["out"].astype(np.float64)
    if bool(int(os.environ.get("BASS_ATTN_DEBUG", "0"))):
        kernel._dbg_results = res.results
    return total.astype(np.float32).reshape(B, T, D)

